# revision 1
# baseline (speedup 1.0000x reference)
"""CRF forward (log-partition) on 8 Trainium2 NeuronCores.

Linear-domain scaled forward algorithm, data-parallel over the batch.

Math: the reference computes, per lane b,
    alpha_0 = onehot-ish(START);  for t < len_b:
    alpha_{t+1}[i] = u_t[i] + logsumexp_j(alpha_t[j] + tr[i, j])
    logZ = logsumexp_i(alpha_len[i] + tr[END, i])
In probability space (p = exp(alpha)) each step is
    p_{t+1} = (E @ p_t) * exp(u_t),   E = exp(tr)
a tiny [64,64] matmul plus an elementwise multiply — ideal for the PE
(stationary weights) + vector engine. Per-lane sequence lengths and the
initial state are folded into a host-prepped, right-aligned log-unary
tensor with one extra "reset" tag, so the device runs one uniform
unconditional 512-step chain for all lanes:
  - warmup steps (t < T-len-1): unary rows = NEG (exp -> 0), reset row = 0
    (exp -> 1): the reset lane carries r=1, real tags stay dead.
  - injection step t* = T-len-1: unary rows = 0, reset row = NEG: the
    matrix column for the reset tag injects onehot(START); r dies.
  - real steps: the lane's actual unaries, shifted by -ln(kappa) per step
    to keep p magnitudes centered in f32 range (measured drift stays
    within e^[-20, 10]); tr[END, :] is added at the final step.
The device streams the 512-step chain; the final state p_T [65, 128] is
DMA'd out and logZ = ln(sum_j p_T[j]) + len * ln(kappa) applied on host.
"""

import os
import sys

import numpy as np

for _p in ("/opt/trn_rl_repo", "/root/.axon_site/_ro/trn_rl_repo"):
    if os.path.isdir(_p) and _p not in sys.path:
        sys.path.append(_p)

import contextlib

import concourse.bacc as bacc
import concourse.bass as bass
import concourse.bass_utils as bass_utils
import concourse.tile as tile
from concourse import mybir
from concourse.bass_utils import run_bass_kernel_spmd


@contextlib.contextmanager
def _walrus_ldw_opt():
    """Compile this kernel with walrus LDWEIGHTS elision enabled.

    The stationary matrix here never changes, so the 512+ per-matmul weight
    reloads (~172 ns each, ~30% of PE time) are pure waste; ldw-opt removes
    the redundant ones. concourse pins --enable-ldw-opt=false globally, so
    swap the flag just for this kernel's compile."""
    orig = bass_utils.run_command

    def patched(argv, **kwargs):
        argv = [
            a.replace("--enable-ldw-opt=false", "--enable-ldw-opt=true")
            if isinstance(a, str)
            else a
            for a in argv
        ]
        return orig(argv, **kwargs)

    bass_utils.run_command = patched
    try:
        yield
    finally:
        bass_utils.run_command = orig

T = 512
N = 64  # tags
NA = N + 1  # + reset tag
BL = 128  # batch lanes per core
NCORES = 8
START_IDX = 1
END_IDX = 2
NEG = -100.0  # exp(NEG) == 0 in f32 up to a ~1e-44 residue that the math kills
LNK = 5.113338285898717  # mean per-step log-growth of the partition mass
GRP = 8  # timesteps per DMA/exp tile
F32 = mybir.dt.float32
F32R = mybir.dt.float32r  # single-pass PE matmul dtype (plain fp32 lowers
# to a HI/LO pass pair at ~4x the cost); ~19-bit storage is plenty here


def _build_program(trace: bool = False):
    nc = bacc.Bacc("TRN2", target_bir_lowering=False, debug=False)
    up_d = nc.dram_tensor("up", [NA, T, BL], F32, kind="ExternalInput")
    # w (stationary matrix) and p0 (initial state) fused into one tensor so
    # the first matmul depends on a single DMA semaphore (PE HW allows only
    # one sync-wait per matmul).
    init_d = nc.dram_tensor("init", [NA, NA + BL], F32R, kind="ExternalInput")
    out_d = nc.dram_tensor("out", [NA, BL], F32R, kind="ExternalOutput")

    HB = BL // 2  # two independent half-chains per core so PE matmuls of one
    # chain overlap the DVE multiply of the other (the per-step serial
    # MM -> sem -> TT -> sem loop otherwise leaves both engines half idle)
    with tile.TileContext(nc) as tc:
        with (
            tc.tile_pool(name="singles", bufs=1) as singles,
            tc.tile_pool(name="upa", bufs=6) as up_pool_a,
            tc.tile_pool(name="upb", bufs=6) as up_pool_b,
            tc.tile_pool(name="ea", bufs=6) as e_pool_a,
            tc.tile_pool(name="eb", bufs=6) as e_pool_b,
            tc.tile_pool(name="pa", bufs=4) as p_pool_a,
            tc.tile_pool(name="pb", bufs=4) as p_pool_b,
            tc.tile_pool(name="za", bufs=4, space="PSUM") as z_pool_a,
            tc.tile_pool(name="zb", bufs=4, space="PSUM") as z_pool_b,
        ):
            init_sb = singles.tile([NA, NA + BL], F32R)
            nc.sync.dma_start(out=init_sb, in_=init_d[:, :])
            w_sb = init_sb[:, 0:NA]
            p_pools = (p_pool_a, p_pool_b)
            z_pools = (z_pool_a, z_pool_b)
            p_cur = [init_sb[:, NA + h * HB : NA + (h + 1) * HB] for h in range(2)]

            up_pools = (up_pool_a, up_pool_b)
            e_pools = (e_pool_a, e_pool_b)
            for g in range(T // GRP):
                e_sbs = []
                # per-half DMA + exp so neither chain's multiply gates on the
                # other chain's unary pipeline at group boundaries
                for h in range(2):
                    up_sb = up_pools[h].tile([NA, GRP, HB], F32, tag=f"up{h}")
                    nc.sync.dma_start(
                        out=up_sb,
                        in_=up_d[:, g * GRP : (g + 1) * GRP, h * HB : (h + 1) * HB],
                    )
                    e_sb = e_pools[h].tile([NA, GRP, HB], F32, tag=f"e{h}")
                    nc.scalar.activation(
                        e_sb, up_sb, mybir.ActivationFunctionType.Exp
                    )
                    e_sbs.append(e_sb)
                for k in range(GRP):
                    for h in range(2):
                        z = z_pools[h].tile([NA, HB], F32, tag=f"z{h}")
                        nc.tensor.matmul(z, w_sb, p_cur[h], start=True, stop=True)
                        p_new = p_pools[h].tile([NA, HB], F32R, tag=f"p{h}")
                        nc.vector.tensor_mul(p_new, z, e_sbs[h][:, k, :])
                        p_cur[h] = p_new

            for h in range(2):
                nc.sync.dma_start(
                    out=out_d[:, h * HB : (h + 1) * HB], in_=p_cur[h]
                )
    nc.compile()
    return nc


def _build_core_inputs(u_core: np.ndarray, len_core: np.ndarray, tr: np.ndarray):
    """u_core [BL, T, N] f32, len_core [BL] -> up [NA, T, BL], p0 [NA, BL]."""
    up = np.full((NA, T, BL), NEG, dtype=np.float32)
    p0 = np.zeros((NA, BL), dtype=np.float32)
    for b in range(BL):
        length = int(len_core[b])
        tstar = T - length - 1
        if length == T:
            p0[START_IDX, b] = 1.0
        else:
            p0[N, b] = 1.0
            up[N, :tstar, b] = 0.0
            up[:N, tstar, b] = 0.0
        up[:N, tstar + 1 :, b] = u_core[b, :length, :].T - LNK
    up[:N, T - 1, :] += tr[END_IDX][:, None]
    return up, p0


def _build_w(tr: np.ndarray) -> np.ndarray:
    w = np.zeros((NA, NA), dtype=np.float32)
    w[:N, :N] = np.exp(tr.astype(np.float32)).T  # lhsT[j, i] = exp(tr[i, j])
    w[N, START_IDX] = 1.0  # injection column
    w[N, N] = 1.0  # reset lane survives (until its unary row kills it)
    return w


def kernel(unary: np.ndarray, trans: np.ndarray, lengths: np.ndarray) -> np.ndarray:
    unary = np.asarray(unary, dtype=np.float32)  # [B, T, N]
    tr = np.asarray(trans, dtype=np.float32)[0]  # [N, N]
    lens = np.asarray(lengths).astype(np.int64)  # [B]
    B = unary.shape[0]
    assert unary.shape == (B, T, N) and B == NCORES * BL

    w = _build_w(tr)
    in_maps = []
    for c in range(NCORES):
        sl = slice(c * BL, (c + 1) * BL)
        up, p0 = _build_core_inputs(unary[sl], lens[sl], tr)
        init = np.concatenate([w, p0], axis=1)  # [NA, NA + BL]
        in_maps.append({"up": up, "init": init})

    nc = _build_program()
    with _walrus_ldw_opt():
        res = run_bass_kernel_spmd(nc, in_maps, list(range(NCORES)))
    sums = np.concatenate(
        [res.results[c]["out"].astype(np.float64).sum(axis=0) for c in range(NCORES)]
    )
    out = np.log(sums.astype(np.float64)) + lens.astype(np.float64) * LNK
    return out.astype(np.float32)



# revision 6
# speedup vs baseline: 2.2690x; 2.2690x over previous
"""CRF forward (log-partition) on 8 Trainium2 NeuronCores.

Bidirectional (meet-in-the-middle) scaled forward algorithm, data-parallel
over the batch, halving the serial chain from 512 to 256 device steps.

Math: logZ_b = ln( f^T (D_{n-1}E) ... (D_0 E) p_0 ),  D_t = diag(exp(u_t)),
E = exp(tr), p_0 = onehot(START), f = exp(tr[END,:]).  Split at m = ceil(n/2):
  forward  : p_{k+1} = e_k   o (E   p_k),   p_0  = onehot(START)   [m steps]
  backward : y_{s+1} = c_s+1 o (E^T y_s),   y_0  = e_{n-1} o f,
             c_s = e_{n-1-s}; a final all-ones step yields q = E^T y.
  logZ = ln(p_m . q_{n-m}) + n*ln(kappa).
Both directions share one block-diagonal [128,128] bf16 stationary
(rows 0:64 fwd tags with E, rows 64:128 bwd tags with E^T), so each device
step is ONE matmul + ONE elementwise multiply for 64 lanes x 2 directions.
The 292us baseline was latency-bound (512 serial PE->DVE->PE round trips of
~570ns); halving the chain and cutting the matmul from f32r (4 cyc/row at
free=64) to bf16 (1 cyc/row) attacks exactly that serial latency.

Emissions exp(u - ln kappa) are precomputed on host in bf16 (halves DMA,
frees the ACT engine).  Per-step states are written into 16-step staging
tiles and DMA'd out in batches (SP sequencer costs ~565ns per DMA issue);
the host picks each lane's split-point states and does the final dot in f64.
Lanes beyond their half-length get zero emissions (state decays to 0 -
multiplicative chain, no NaNs), so no hold/injection machinery is needed.
"""

import os
import sys

import numpy as np

for _p in ("/opt/trn_rl_repo", "/root/.axon_site/_ro/trn_rl_repo"):
    if os.path.isdir(_p) and _p not in sys.path:
        sys.path.append(_p)

import contextlib

import ml_dtypes

import concourse.bacc as bacc
import concourse.bass_utils as bass_utils
import concourse.tile as tile
from concourse import mybir
from concourse.bass_utils import run_bass_kernel_spmd


@contextlib.contextmanager
def _walrus_ldw_opt():
    """No-op (kept for the test harness API).

    bf16 LDWEIGHTS is rejected by the walrus ldw-opt pass, and mixed
    f32r/bf16 matmul inputs are rejected by the verifier, so the kernel
    runs all-bf16 with the default per-matmul weight reloads: a [128,128]
    bf16 LDWEIGHTS (~55ns) has no data deps and hides inside the
    PE-to-DVE semaphore wait gap of the serial chain."""
    yield


T = 512
T2 = 256  # bidirectional: device chain length
N = 64  # tags
N2 = 128  # fwd tags + bwd tags stacked on partitions
BL = 128  # batch lanes per core
HB = BL // 2  # lanes per chain (2 chains overlap PE with the mul engines)
NCORES = 8
START_IDX = 1
END_IDX = 2
LNK = 5.113338285898717  # mean per-step log-growth of the partition mass
GRP = 16  # timesteps per DMA/staging tile
BF16 = mybir.dt.bfloat16
F32 = mybir.dt.float32



def _build_program(trace: bool = False):
    nc = bacc.Bacc("TRN2", target_bir_lowering=False, debug=False)
    e2_d = nc.dram_tensor("e2", [N2, T2, BL], BF16, kind="ExternalInput")
    # stationary W2 and initial states fused: first matmul then depends on a
    # single DMA semaphore (PE HW allows only one sync-wait per matmul).
    init_d = nc.dram_tensor("init", [N2, N2 + BL], BF16, kind="ExternalInput")
    out_d = nc.dram_tensor("out", [N2, T2, BL], BF16, kind="ExternalOutput")

    with tile.TileContext(nc) as tc:
        with (
            tc.tile_pool(name="singles", bufs=1) as singles,
            tc.tile_pool(name="ea", bufs=3) as e_pool_a,
            tc.tile_pool(name="eb", bufs=3) as e_pool_b,
            tc.tile_pool(name="sta", bufs=3) as st_pool_a,
            tc.tile_pool(name="stb", bufs=3) as st_pool_b,
            tc.tile_pool(name="za", bufs=4, space="PSUM") as z_pool_a,
            tc.tile_pool(name="zb", bufs=4, space="PSUM") as z_pool_b,
        ):
            init_sb = singles.tile([N2, N2 + BL], BF16)
            nc.sync.dma_start(out=init_sb, in_=init_d[:, :])
            w_sb = init_sb[:, 0:N2]
            e_pools = (e_pool_a, e_pool_b)
            st_pools = (st_pool_a, st_pool_b)
            z_pools = (z_pool_a, z_pool_b)
            mul_engines = (nc.vector, nc.vector)  # GPSIMD cannot access PSUM
            p_cur = [init_sb[:, N2 + h * HB : N2 + (h + 1) * HB] for h in range(2)]

            for g in range(T2 // GRP):
                e_sbs, st_sbs = [], []
                for h in range(2):
                    e_sb = e_pools[h].tile([N2, GRP, HB], BF16, tag=f"e{h}")
                    nc.sync.dma_start(
                        out=e_sb,
                        in_=e2_d[:, g * GRP : (g + 1) * GRP, h * HB : (h + 1) * HB],
                    )
                    e_sbs.append(e_sb)
                    st_sb = st_pools[h].tile([N2, GRP, HB], BF16, tag=f"st{h}")
                    st_sbs.append(st_sb)
                for k in range(GRP):
                    for h in range(2):
                        z = z_pools[h].tile([N2, HB], F32, tag=f"z{h}")
                        nc.tensor.matmul(z, w_sb, p_cur[h], start=True, stop=True)
                        p_new = st_sbs[h][:, k, :]
                        mul_engines[h].tensor_mul(p_new, z, e_sbs[h][:, k, :])
                        p_cur[h] = p_new
                for h in range(2):
                    nc.sync.dma_start(
                        out=out_d[
                            :, g * GRP : (g + 1) * GRP, h * HB : (h + 1) * HB
                        ],
                        in_=st_sbs[h],
                    )
    nc.compile()
    return nc


def _split_lengths(lens: np.ndarray):
    mf = (lens + 1) // 2  # forward steps, 1..256
    mb = lens - mf  # backward device steps (incl. final ones-step), 0..256
    return mf, mb


def _build_core_inputs(
    u_core: np.ndarray, len_core: np.ndarray, E: np.ndarray
) -> dict:
    """u_core [BL, T, N] f32, len_core [BL] -> e2 [N2, T2, BL], init [N2, N2+BL]."""
    f = E[END_IDX]  # exp(tr[END, :])
    ex = np.exp(u_core.astype(np.float64) - LNK).astype(np.float32)  # [BL,T,N]
    mf, mb = _split_lengths(len_core)

    e2 = np.zeros((N2, T2, BL), dtype=np.float32)
    d = np.arange(T2)
    # forward rows: slot d = ex[b, d, :] while d < mf
    fwd = np.where(
        (d[None, :, None] < mf[:, None, None]), ex[:, :T2, :], 0.0
    )  # [BL, T2, N]
    e2[0:N] = fwd.transpose(2, 1, 0)
    # backward rows: slot d = ex[b, n-2-d, :] for d <= mb-2; slot mb-1 = ones
    idx = np.clip(len_core[:, None] - 2 - d[None, :], 0, T - 1)  # [BL, T2]
    bwd = np.take_along_axis(ex, idx[:, :, None], axis=1)  # [BL, T2, N]
    bwd = np.where((d[None, :, None] <= mb[:, None, None] - 2), bwd, 0.0)
    ones_slot = d[None, :, None] == (mb[:, None, None] - 1)
    bwd = np.where(ones_slot, 1.0, bwd)
    e2[N:N2] = bwd.transpose(2, 1, 0)

    # initial states: fwd p0 = onehot(START); bwd y0 = e_{n-1} o f (if mb>=1)
    s0 = np.zeros((N2, BL), dtype=np.float32)
    s0[START_IDX, :] = 1.0
    elast = np.take_along_axis(
        ex, (len_core[:, None, None] - 1).astype(np.int64), axis=1
    )[:, 0, :]  # [BL, N] = ex[b, n-1, :]
    y0 = (elast * f[None, :]) * (mb > 0)[:, None]
    s0[N:N2] = y0.T

    # stationary: lhsT[j, i] = E[i, j] (fwd block); lhsT[64+j, 64+i] = E[j, i]
    w2 = np.zeros((N2, N2), dtype=np.float32)
    w2[0:N, 0:N] = E.T
    w2[N:N2, N:N2] = E
    init = np.concatenate([w2, s0], axis=1)  # [N2, N2 + BL]
    return {
        "e2": e2.astype(ml_dtypes.bfloat16),
        "init": init.astype(ml_dtypes.bfloat16),
    }


def _build_in_maps(unary: np.ndarray, tr: np.ndarray, lens: np.ndarray):
    E = np.exp(tr.astype(np.float64)).astype(np.float32)
    in_maps = []
    for c in range(NCORES):
        sl = slice(c * BL, (c + 1) * BL)
        in_maps.append(_build_core_inputs(unary[sl], lens[sl], E))
    return in_maps


def _postprocess(results, tr: np.ndarray, lens: np.ndarray) -> np.ndarray:
    E = np.exp(tr.astype(np.float64))
    f = E[END_IDX]  # [N]
    mf, mb = _split_lengths(lens)
    out = np.empty(lens.shape[0], dtype=np.float64)
    for c in range(NCORES):
        res = np.asarray(results[c]["out"]).astype(np.float64)  # [N2, T2, BL]
        for b in range(BL):
            gb = c * BL + b
            p = res[0:N, mf[gb] - 1, b]
            q = res[N:N2, mb[gb] - 1, b] if mb[gb] >= 1 else f
            out[gb] = np.log(np.dot(p, q)) + lens[gb] * LNK
    return out.astype(np.float32)


def kernel(unary: np.ndarray, trans: np.ndarray, lengths: np.ndarray) -> np.ndarray:
    unary = np.asarray(unary, dtype=np.float32)  # [B, T, N]
    tr = np.asarray(trans, dtype=np.float32)[0]  # [N, N]
    lens = np.asarray(lengths).astype(np.int64)  # [B]
    B = unary.shape[0]
    assert unary.shape == (B, T, N) and B == NCORES * BL

    in_maps = _build_in_maps(unary, tr, lens)
    nc = _build_program()
    with _walrus_ldw_opt():
        res = run_bass_kernel_spmd(nc, in_maps, list(range(NCORES)))
    return _postprocess(res.results, tr, lens)
